# revision 22
# baseline (speedup 1.0000x reference)
# DeepseekV2 MLA attention (T=2048, H=16) on 8 TRN2 NeuronCores.
#
# Two launches (host gather/transpose/normalize between them is free):
#   Launch 1 (seq x col 2D, 4 seq-blocks x 2 weight-col-halves): each core
#     computes raw low-rank latents for its 512-token block and weight-column
#     half.  RMSNorm (incl. sum-of-squares from the bf16 latents) + k_pe rope
#     are applied on the host (elementwise, cheap).
#   Launch 2 (head-parallel, 2 heads/core): up-projections, q-rope, causal
#     softmax attention (scores kept [k, q]; denominator via DVE column-sum
#     accumulation + one fp32 broadcast matmul), o_proj partial in bf16;
#     host sums the 8 partials.
#
# Perf notes vs the naive version:
#   - all DRAM tensors host-packed so each DMA descriptor covers >=2KB/partition
#     (the 16 DMA queues are descriptor-rate-bound at ~95ns/descriptor)
#   - PE kept continuously busy (TRN2 PE needs ~3us of back-to-back work to
#     reach the 2.4GHz p-state)
#   - no per-tile softmax-denominator matmuls; no 1-partition reciprocals
#   - causal diagonal tiles narrowed to valid columns
#   - both heads' rope projection packed into one stationary; rope applied by
#     DVE with a DMA partition-block swap
#   - RMSNorm weights folded into wq_b / wkv_b on the host
import contextlib
import ctypes
import math
import sys
import types

import numpy as np

# ---------------------------------------------------------------- constants
H = 16
D_NOPE = 128
D_ROPE = 64
D_QK = D_NOPE + D_ROPE
D_V = 128
HID = 2048
Q_RANK = 1536
KV_RANK = 512
EPS = 1e-6
T = 2048
BASE = 10000.0
FACTOR = 40.0
ORIG_MAX = 4096
BETA_FAST = 32.0
BETA_SLOW = 1.0
MSCALE = 0.707
MSCALE_ALL = 0.707

N_CORES = 8
SEQB = 4                    # launch-1 sequence blocks
TC1 = T // SEQB             # 512 tokens per launch-1 core
WQH = Q_RANK // 2           # 768 wq_a columns per half
WKH = (KV_RANK + D_ROPE) // 2  # 288 wkv_a columns per half
HPC = H // N_CORES          # 2 heads per launch-2 core
QT = 512                    # q-tile width
KT = 128                    # k-tile height
RCH = Q_RANK // 128         # 12
KVCH = KV_RANK // 128       # 4
KCH = HID // 128            # 16


def _yarn_mscale(scale, mscale):
    return 1.0 if scale <= 1 else 0.1 * mscale * math.log(scale) + 1.0


SCALING = D_QK ** -0.5 * _yarn_mscale(FACTOR, MSCALE_ALL) ** 2

# ------------------------------------------------------- NTFF profiling shim
LAST_EXEC_NS = []


def _install_ntff_shim():
    try:
        import antenv.axon_hooks  # noqa: F401
        return
    except ImportError:
        pass
    try:
        so_path = "/opt/axon/libaxon_pjrt.so"
        lib = ctypes.CDLL(so_path)
        if not hasattr(lib, "axon_start_nrt_profile"):
            hook = None
        else:
            lib.axon_start_nrt_profile.argtypes = [
                ctypes.POINTER(ctypes.c_int64),
                ctypes.c_size_t,
            ]
            lib.axon_start_nrt_profile.restype = ctypes.c_int64
            lib.axon_stop_nrt_profile.argtypes = [ctypes.c_char_p]
            lib.axon_stop_nrt_profile.restype = ctypes.c_int64

            @contextlib.contextmanager
            def hook(output_dir, device_ids):
                import jax

                jax.devices()
                if device_ids:
                    ids = (ctypes.c_int64 * len(device_ids))(*device_ids)
                    rc = lib.axon_start_nrt_profile(ids, len(device_ids))
                else:
                    rc = lib.axon_start_nrt_profile(None, 0)
                if rc != 0:
                    raise RuntimeError(f"axon_start_nrt_profile rc={rc}")
                try:
                    yield
                finally:
                    n = lib.axon_stop_nrt_profile(str(output_dir).encode())
                    if n < 0:
                        raise RuntimeError(f"axon_stop_nrt_profile rc={n}")

        mod = types.ModuleType("antenv.axon_hooks")
        mod.get_axon_ntff_profile_hook = lambda: hook
        mod.set_axon_ntff_profile_hook = lambda h: None
        sys.modules["antenv.axon_hooks"] = mod
    except Exception:
        pass


_install_ntff_shim()

# ------------------------------------------------------------- host helpers


def _rope_tables(positions):
    dim = D_ROPE
    pos_freqs = BASE ** (np.arange(0, dim, 2, dtype=np.float64) / dim)
    inv_extra = 1.0 / pos_freqs
    inv_inter = 1.0 / (FACTOR * pos_freqs)

    def corr(nr):
        return dim * math.log(ORIG_MAX / (nr * 2 * math.pi)) / (2 * math.log(BASE))

    low = max(math.floor(corr(BETA_FAST)), 0)
    high = min(math.ceil(corr(BETA_SLOW)), dim - 1)
    ramp = np.clip(
        (np.arange(dim // 2, dtype=np.float64) - low) / max(high - low, 0.001), 0.0, 1.0
    )
    mask = 1.0 - ramp
    inv_freq = inv_inter * (1.0 - mask) + inv_extra * mask
    freqs = np.outer(np.asarray(positions, np.float64), inv_freq)
    m = _yarn_mscale(FACTOR, MSCALE) / _yarn_mscale(FACTOR, MSCALE_ALL)
    return (np.cos(freqs) * m).astype(np.float32), (np.sin(freqs) * m).astype(np.float32)


# ------------------------------------------------------------ bass builders
_BUILD_CACHE = {}


def _build_phase1():
    from concourse import bacc, mybir
    from concourse.tile import TileContext

    F32 = mybir.dt.float32
    BF16 = mybir.dt.bfloat16

    MT1 = TC1 // 128
    nc = bacc.Bacc()
    hTp = nc.dram_tensor("hTp", [128, KCH, TC1], BF16, kind="ExternalInput")
    wqap = nc.dram_tensor("wqap", [128, KCH, WQH], BF16, kind="ExternalInput")
    wkvap = nc.dram_tensor("wkvap", [128, KCH, WKH], BF16, kind="ExternalInput")
    qa_out = nc.dram_tensor("qa", [128, MT1, WQH], BF16, kind="ExternalOutput")
    kva_out = nc.dram_tensor("kva", [128, MT1, WKH], BF16, kind="ExternalOutput")

    MT = TC1 // 128  # 4 token tiles

    with TileContext(nc) as tc, contextlib.ExitStack() as ctx:
        pool = ctx.enter_context(tc.tile_pool(name="sb", bufs=1))
        work = ctx.enter_context(tc.tile_pool(name="wk", bufs=2))

        hT_sb = pool.tile([128, KCH, TC1], BF16, tag="hT")
        wqa_sb = pool.tile([128, KCH, WQH], BF16, tag="wqa")
        wkva_sb = pool.tile([128, KCH, WKH], BF16, tag="wkva")
        qa_st = pool.tile([128, MT, WQH], BF16, tag="qast")
        kva_st = pool.tile([128, MT, WKH], BF16, tag="kvast")
        ones1 = pool.tile([128, 128], BF16, tag="ones1")
        nc.vector.memset(ones1[:, :], 1.0)

        # loads in consumption order; >=2KB/partition descriptors.
        # spread across engine rings for more in-flight descriptors
        def ld(out, in_):
            nc.sync.dma_start(out=out, in_=in_)

        for g in range(4):
            ks = slice(4 * g, 4 * g + 4)
            ld(hT_sb[:, ks, :], hTp[:, ks, :])
            ld(wqa_sb[:, ks, :], wqap[:, ks, :])
            if g in (1, 2):
                ks2 = slice(8 * (g - 1), 8 * (g - 1) + 8)
                ld(wkva_sb[:, ks2, :], wkvap[:, ks2, :])

        # qa + kva chains share one PSUM scope (4 + 4 banks) so there is no
        # pool barrier between the stages and the PE never goes idle
        with tc.tile_pool(name="ppq", bufs=1, space="PSUM") as ppq, \
             tc.tile_pool(name="ppk", bufs=2, space="PSUM") as ppk:
            # keep the PE busy/ramping while the first input DMAs land
            warm = ppq.tile([128, WQH], F32, tag="qa0", name="warm")
            for wi in range(16):
                nc.tensor.matmul(warm[:, 0:128], ones1[:, :], ones1[:, :],
                                 start=True, stop=True)
            for mg in range(2):
                qa_ps = [ppq.tile([128, WQH], F32, tag=f"qa{mi}",
                                  name=f"qa{mg}_{mi}") for mi in range(2)]
                for k in range(KCH):
                    for mi in range(2):
                        m = 2 * mg + mi
                        stat = hT_sb[:, k, m * 128:(m + 1) * 128]
                        nc.tensor.matmul(qa_ps[mi][:, 0:512], stat,
                                         wqa_sb[:, k, 0:512],
                                         start=(k == 0), stop=(k == KCH - 1))
                        nc.tensor.matmul(qa_ps[mi][:, 512:WQH], stat,
                                         wqa_sb[:, k, 512:WQH],
                                         start=(k == 0), stop=(k == KCH - 1))
                for mi in range(2):
                    m = 2 * mg + mi
                    nc.vector.tensor_copy(qa_st[:, m, :], qa_ps[mi][:, :])
                nc.sync.dma_start(out=qa_out[:, 2 * mg:2 * mg + 2, :],
                                  in_=qa_st[:, 2 * mg:2 * mg + 2, :])
            for mg in range(2):
                kv_ps = [ppk.tile([128, WKH], F32, tag=f"kv{mi}",
                                  name=f"kv{mg}_{mi}") for mi in range(2)]
                for k in range(KCH):
                    for mi in range(2):
                        m = 2 * mg + mi
                        stat = hT_sb[:, k, m * 128:(m + 1) * 128]
                        nc.tensor.matmul(kv_ps[mi][:, :], stat, wkva_sb[:, k, :],
                                         start=(k == 0), stop=(k == KCH - 1))
                for mi in range(2):
                    m = 2 * mg + mi
                    nc.vector.tensor_copy(kva_st[:, m, :], kv_ps[mi][:, :])
                nc.sync.dma_start(out=kva_out[:, 2 * mg:2 * mg + 2, :],
                                  in_=kva_st[:, 2 * mg:2 * mg + 2, :])

    nc.finalize()
    return nc


def _build_phase2():
    from concourse import bacc, mybir
    from concourse.tile import TileContext

    F32 = mybir.dt.float32
    BF16 = mybir.dt.bfloat16
    FP16 = mybir.dt.float16
    AF = mybir.ActivationFunctionType
    OP = mybir.AluOpType
    EXPB = -8.0 * math.log(2.0)  # exp bias; cancels in softmax, keeps fp16 range

    nc = bacc.Bacc()
    kvaTp = nc.dram_tensor("kvaTp", [128, KVCH, T], BF16, kind="ExternalInput")
    qaTp = nc.dram_tensor("qaTp", [128, RCH, T], BF16, kind="ExternalInput")
    kpe2 = nc.dram_tensor("kpe2", [128, T], BF16, kind="ExternalInput")
    wkbn = nc.dram_tensor("wkbn", [128, KVCH, HPC, 128], BF16, kind="ExternalInput")
    wkbv = nc.dram_tensor("wkbv", [128, KVCH, 256], BF16, kind="ExternalInput")
    wqbn = nc.dram_tensor("wqbn", [128, RCH, HPC, 128], BF16, kind="ExternalInput")
    wqbp = nc.dram_tensor("wqbp", [128, RCH, 128], BF16, kind="ExternalInput")
    wop = nc.dram_tensor("wop", [128, HPC, HID], BF16, kind="ExternalInput")
    csd = nc.dram_tensor("csd", [128, T], BF16, kind="ExternalInput")
    ssd = nc.dram_tensor("ssd", [128, T], BF16, kind="ExternalInput")
    maskd = nc.dram_tensor("maskd", [128, 896], FP16, kind="ExternalInput")
    out_p = nc.dram_tensor("out_p", [T, HID], BF16, kind="ExternalOutput")

    with TileContext(nc) as tc, contextlib.ExitStack() as ctx:
        persist = ctx.enter_context(tc.tile_pool(name="persist", bufs=1))

        kvaT_sb = persist.tile([128, KVCH, T], BF16, tag="kvaT")
        qaT_sb = persist.tile([128, RCH, T], BF16, tag="qaT")
        kpe2_sb = persist.tile([128, T], BF16, tag="kpe2")
        wkbn_sb = persist.tile([128, KVCH, HPC, 128], BF16, tag="wkbn")
        wkbv_sb = persist.tile([128, KVCH, 256], BF16, tag="wkbv")
        wqbn_sb = persist.tile([128, RCH, HPC, 128], BF16, tag="wqbn")
        wqbp_sb = persist.tile([128, RCH, 128], BF16, tag="wqbp")
        wo_sb = persist.tile([128, HPC, HID], BF16, tag="wo")
        cs_sb = persist.tile([128, T], BF16, tag="cs")
        ss_sb = persist.tile([128, T], BF16, tag="ss")
        mask_sb = persist.tile([128, 896], FP16, tag="mask")
        ones_sb = persist.tile([128, 128], FP16, tag="ones")
        nc.vector.memset(ones_sb[:, :], 1.0)
        expb_sb = persist.tile([128, 1], F32, tag="expb")
        nc.vector.memset(expb_sb[:, :], EXPB)

        knopeT = [persist.tile([128, T], BF16, tag=f"knopeT{h}", name=f"knopeT{h}") for h in range(HPC)]
        v_nat = [persist.tile([128, T], FP16, tag=f"vnat{h}", name=f"vnat{h}") for h in range(HPC)]
        qnT = [persist.tile([128, T], BF16, tag=f"qnT{h}", name=f"qnT{h}") for h in range(HPC)]
        qpeT = persist.tile([128, T], BF16, tag="qpeT")  # [h0 x'|y' ; h1 x'|y']
        aoT = [persist.tile([128, T], BF16, tag=f"aoT{h}", name=f"aoT{h}") for h in range(HPC)]
        colsum = [persist.tile([128, QT], FP16, tag=f"colsum{h}", name=f"colsum{h}") for h in range(HPC)]

        # loads: PE-critical order first
        def ld(out, in_):
            nc.sync.dma_start(out=out, in_=in_)

        ld(kvaT_sb[:, 0, :], kvaTp[:, 0, :])
        ld(wkbn_sb[:, :, :, :], wkbn[:, :, :, :])
        ld(wkbv_sb[:, :, :], wkbv[:, :, :])
        for k in range(1, KVCH):
            ld(kvaT_sb[:, k, :], kvaTp[:, k, :])
        ld(wqbn_sb[:, :, :, :], wqbn[:, :, :, :])
        ld(wqbp_sb[:, :, :], wqbp[:, :, :])
        for k in range(RCH):
            ld(qaT_sb[:, k, :], qaTp[:, k, :])
        ld(cs_sb[:, :], csd[:, :])
        ld(ss_sb[:, :], ssd[:, :])
        ld(kpe2_sb[:, :], kpe2[:, :])
        ld(mask_sb[:, :], maskd[:, :])
        ld(wo_sb[:, :, :], wop[:, :, :])

        # ---------------- stage 1a: k_nope^T = wkbn^T kva, k-outer ----------
        with tc.tile_pool(name="ppkn", bufs=1, space="PSUM") as ppkn:
            warm = ppkn.tile([128, 512], F32, tag="kn0_0", name="warm")
            for wi in range(16):
                nc.tensor.matmul(warm[:, 0:128], ones_sb[:, :], ones_sb[:, :],
                                 start=True, stop=True)
            kn_ps = [[ppkn.tile([128, 512], F32, tag=f"kn{h}_{n}", name=f"kn{h}_{n}")
                      for n in range(4)] for h in range(HPC)]
            for k in range(KVCH):
                for h in range(HPC):
                    for n in range(4):
                        nsl = slice(n * 512, (n + 1) * 512)
                        nc.tensor.matmul(
                            kn_ps[h][n][:, :], wkbn_sb[:, k, h, :],
                            kvaT_sb[:, k, nsl],
                            start=(k == 0), stop=(k == KVCH - 1))
            for h in range(HPC):
                for n in range(4):
                    nsl = slice(n * 512, (n + 1) * 512)
                    if n % 2 == 0:
                        nc.scalar.copy(knopeT[h][:, nsl], kn_ps[h][n][:, :])
                    else:
                        nc.vector.tensor_copy(knopeT[h][:, nsl], kn_ps[h][n][:, :])

        # ---------------- stage 1b: v (both heads packed) -------------------
        with tc.tile_pool(name="ppv", bufs=4, space="PSUM") as ppv:
            for t in range(T // 128):
                tsl = slice(t * 128, (t + 1) * 128)
                v_ps = ppv.tile([128, 256], F32, tag="v", name=f"v{t}")
                for k in range(KVCH):
                    nc.tensor.matmul(v_ps[:, :], kvaT_sb[:, k, tsl],
                                     wkbv_sb[:, k, :],
                                     start=(k == 0), stop=(k == KVCH - 1))
                nc.scalar.copy(v_nat[0][:, tsl], v_ps[:, 0:128])
                nc.scalar.copy(v_nat[1][:, tsl], v_ps[:, 128:256])

        # ------------- stage 2: q up-projections + rope (2-qtr groups) ------
        with tc.tile_pool(name="ppg", bufs=2, space="PSUM") as ppg, \
             tc.tile_pool(name="qwork", bufs=2) as qwork:
            for qg in range(2):
                qtrs = [2 * qg, 2 * qg + 1]
                qn_ps = {}
                qp_ps = {}
                for qtr in qtrs:
                    qn_ps[qtr] = [ppg.tile([128, QT], F32, tag=f"qn{h}",
                                           name=f"qn{h}_{qtr}")
                                  for h in range(HPC)]
                    qp_ps[qtr] = ppg.tile([128, QT], F32, tag="qp",
                                          name=f"qp_{qtr}")
                for k in range(RCH):
                    for qtr in qtrs:
                        qsl = slice(qtr * QT, (qtr + 1) * QT)
                        mov = qaT_sb[:, k, qsl]
                        for h in range(HPC):
                            nc.tensor.matmul(qn_ps[qtr][h][:, :],
                                             wqbn_sb[:, k, h, :], mov,
                                             start=(k == 0), stop=(k == RCH - 1))
                        nc.tensor.matmul(qp_ps[qtr][:, :], wqbp_sb[:, k, :], mov,
                                         start=(k == 0), stop=(k == RCH - 1))
                for qtr in qtrs:
                    qsl = slice(qtr * QT, (qtr + 1) * QT)
                    for h in range(HPC):
                        nc.scalar.copy(qnT[h][:, qsl], qn_ps[qtr][h][:, :])
                    # rope: evacuate, block-swap head-halves via DMA, then
                    # qpeT = qp*CS + swap(qp)*SS on DVE
                    qp_sb = qwork.tile([128, QT], F32, tag="qpsb",
                                       name=f"qpsb{qtr}")
                    nc.scalar.copy(qp_sb[:, :], qp_ps[qtr][:, :])
                    sw_sb = qwork.tile([128, QT], F32, tag="swsb",
                                       name=f"swsb{qtr}")
                    nc.gpsimd.dma_start(out=sw_sb[0:32, :], in_=qp_sb[32:64, :])
                    nc.gpsimd.dma_start(out=sw_sb[32:64, :], in_=qp_sb[0:32, :])
                    nc.gpsimd.dma_start(out=sw_sb[64:96, :], in_=qp_sb[96:128, :])
                    nc.gpsimd.dma_start(out=sw_sb[96:128, :], in_=qp_sb[64:96, :])
                    ta = qwork.tile([128, QT], BF16, tag="ta", name=f"ta{qtr}")
                    nc.vector.tensor_tensor(ta[:, :], qp_sb[:, :], cs_sb[:, qsl],
                                            op=OP.mult)
                    tb = qwork.tile([128, QT], BF16, tag="tb", name=f"tb{qtr}")
                    nc.vector.tensor_tensor(tb[:, :], sw_sb[:, :], ss_sb[:, qsl],
                                            op=OP.mult)
                    nc.vector.tensor_tensor(qpeT[:, qsl], ta[:, :], tb[:, :],
                                            op=OP.add)

        # ------- stage 3: attention; softmax tail + o_proj pipelined 1 qtr ----
        with tc.tile_pool(name="pps", bufs=2, space="PSUM") as pps, \
             tc.tile_pool(name="ppu", bufs=3, space="PSUM") as ppu, \
             tc.tile_pool(name="ppd", bufs=1, space="PSUM") as ppd, \
             tc.tile_pool(name="ppo", bufs=2, space="PSUM") as ppo, \
             tc.tile_pool(name="awork", bufs=8) as awork, \
             tc.tile_pool(name="rwork", bufs=2) as rwork, \
             tc.tile_pool(name="owork", bufs=3) as owork:

            o_sb_map = {}

            def emit_o_unit(qtr, tt, j):
                # one o_proj 512-col chunk for token tile tt of q-range qtr
                q0o = qtr * QT
                tslo = slice(q0o + tt * 128, q0o + (tt + 1) * 128)
                jsl = slice(j * 512, (j + 1) * 512)
                if j == 0:
                    o_sb_map[(qtr, tt)] = owork.tile(
                        [128, HID], BF16, tag="osb", name=f"o{qtr}_{tt}")
                o_sb = o_sb_map[(qtr, tt)]
                o_ps = ppo.tile([128, 512], F32, tag="o",
                                name=f"op{qtr}_{tt}_{j}")
                for h in range(HPC):
                    nc.tensor.matmul(o_ps[:, :], aoT[h][:, tslo],
                                     wo_sb[:, h, jsl],
                                     start=(h == 0), stop=(h == HPC - 1))
                if j % 2 == 0:
                    nc.scalar.copy(o_sb[:, jsl], o_ps[:, :])
                else:
                    nc.vector.tensor_copy(o_sb[:, jsl], o_ps[:, :])
                if j == 3:
                    nc.sync.dma_start(out=out_p[tslo, :], in_=o_sb[:, :])

            def make_tail(qtr, h, un_t):
                def emit():
                    qslh = slice(qtr * QT, (qtr + 1) * QT)
                    denb_ps = ppd.tile([128, QT], F32, tag="denb",
                                       name=f"db{h}_{qtr}")
                    nc.tensor.matmul(denb_ps[:, :], ones_sb[:, :],
                                     colsum[h][:, :], start=True, stop=True)
                    recip = rwork.tile([128, QT], F32, tag="recip",
                                       name=f"r{h}_{qtr}")
                    nc.vector.reciprocal_approx_fast(out=recip[:, :],
                                                     in_=denb_ps[:, :])
                    nc.vector.tensor_tensor(aoT[h][:, qslh], un_t[:, :],
                                            recip[:, :], op=OP.mult)
                return emit

            warm = pps.tile([128, QT], F32, tag="s", name="warm")
            for wi in range(24):
                nc.tensor.matmul(warm[:, 0:128], ones_sb[:, :], ones_sb[:, :],
                                 start=True, stop=True)
            filler = []
            for qtr in range(T // QT):
                q0 = qtr * QT
                n_k = (q0 + QT) // KT
                un_ps = [None] * HPC
                for ki in range(n_k):
                    k0 = ki * KT
                    ksl = slice(k0, k0 + KT)
                    d = k0 - q0
                    coff = max(d, 0)
                    w = QT - coff
                    msl = slice(q0 + coff, q0 + QT)
                    for h in range(HPC):
                        if filler:
                            filler.pop(0)()
                        if ki == 0:
                            un_ps[h] = ppu.tile([128, QT], F32, tag="un",
                                                name=f"un{h}_{qtr}")
                        hb = slice(64 * h, 64 * h + 64)
                        s_ps = pps.tile([128, QT], F32, tag="s",
                                        name=f"s{h}_{qtr}_{ki}")
                        nc.tensor.matmul(s_ps[:, 0:w], knopeT[h][:, ksl],
                                         qnT[h][:, msl], start=True, stop=False)
                        nc.tensor.matmul(s_ps[:, 0:w], kpe2_sb[hb, ksl],
                                         qpeT[hb, msl], start=False, stop=True)
                        expT = awork.tile([128, QT], FP16, tag="expT",
                                          name=f"e{h}_{qtr}_{ki}")
                        nc.scalar.activation(out=expT[:, 0:w], in_=s_ps[:, 0:w],
                                             func=AF.Exp, scale=SCALING,
                                             bias=expb_sb[:, :])
                        if d >= 0:
                            nc.vector.tensor_tensor(
                                expT[:, 0:w], expT[:, 0:w],
                                mask_sb[:, 384:384 + w], op=OP.mult)
                        if ki == 0:
                            nc.vector.tensor_copy(colsum[h][:, :], expT[:, :])
                        else:
                            nc.vector.tensor_tensor(
                                colsum[h][:, coff:QT], colsum[h][:, coff:QT],
                                expT[:, 0:w], op=OP.add)
                        nc.tensor.matmul(un_ps[h][:, coff:QT], v_nat[h][:, ksl],
                                         expT[:, 0:w],
                                         start=(ki == 0), stop=(ki == n_k - 1),
                                         skip_group_check=True)
                while filler:
                    filler.pop(0)()
                filler = [make_tail(qtr, h, un_ps[h]) for h in range(HPC)]
                filler += [(lambda a, b, c: (lambda: emit_o_unit(a, b, c)))(
                    qtr, tt, j) for tt in range(QT // 128) for j in range(4)]
            while filler:
                filler.pop(0)()

    nc.finalize()
    return nc


def _get_built(name):
    if name not in _BUILD_CACHE:
        _BUILD_CACHE[name] = _build_phase1() if name == "p1" else _build_phase2()
    return _BUILD_CACHE[name]


# ---------------------------------------------------------------- kernel()


def kernel(positions, hidden_states, wq_a, q_a_norm_w, wq_b, wkv_a, kv_a_norm_w,
           wkv_b, wo):
    import os

    from concourse.bass_utils import run_bass_kernel_spmd
    import ml_dtypes

    BFNP = ml_dtypes.bfloat16
    trace = bool(os.environ.get("BASS_KERNEL_TRACE"))
    LAST_EXEC_NS.clear()

    positions = np.asarray(positions)
    hidden = np.asarray(hidden_states, np.float32)
    wq_a = np.asarray(wq_a, np.float32)
    wq_b = np.asarray(wq_b, np.float32)
    wkv_a = np.asarray(wkv_a, np.float32)
    wkv_b = np.asarray(wkv_b, np.float32)
    wo = np.asarray(wo, np.float32)
    q_a_norm_w = np.asarray(q_a_norm_w, np.float32)
    kv_a_norm_w = np.asarray(kv_a_norm_w, np.float32)

    cos, sin = _rope_tables(positions)  # [T, 32] f32

    # ---------------- launch 1: latents (4 seq blocks x 2 col halves) -------
    hidden_bf = hidden.astype(BFNP)
    wqa_halves = []
    wkva_halves = []
    for half in range(2):
        wq_h = wq_a[:, half * WQH:(half + 1) * WQH].astype(BFNP)
        wqa_halves.append(np.ascontiguousarray(
            wq_h.reshape(KCH, 128, WQH).transpose(1, 0, 2)))
        wk_h = wkv_a[:, half * WKH:(half + 1) * WKH].astype(BFNP)
        wkva_halves.append(np.ascontiguousarray(
            wk_h.reshape(KCH, 128, WKH).transpose(1, 0, 2)))

    in_maps1 = []
    for c in range(N_CORES):
        seq, half = c // 2, c % 2
        hs = hidden_bf[seq * TC1:(seq + 1) * TC1]  # [512, 2048]
        hTp = np.ascontiguousarray(hs.reshape(TC1, KCH, 128).transpose(2, 1, 0))
        in_maps1.append({
            "hTp": hTp,
            "wqap": wqa_halves[half],
            "wkvap": wkva_halves[half],
        })

    nc1 = _get_built("p1")
    res1 = run_bass_kernel_spmd(nc1, in_maps1, core_ids=list(range(N_CORES)),
                                trace=trace)
    if trace:
        LAST_EXEC_NS.append(res1.exec_time_ns)

    qa_full = np.empty((T, Q_RANK), np.float32)
    kva_full = np.empty((T, KV_RANK + D_ROPE), np.float32)
    for c in range(N_CORES):
        seq, half = c // 2, c % 2
        r = res1.results[c]
        tsl = slice(seq * TC1, (seq + 1) * TC1)
        qa_full[tsl, half * WQH:(half + 1) * WQH] = (
            np.asarray(r["qa"], np.float32).transpose(1, 0, 2).reshape(TC1, WQH))
        kva_full[tsl, half * WKH:(half + 1) * WKH] = (
            np.asarray(r["kva"], np.float32).transpose(1, 0, 2).reshape(TC1, WKH))

    # host RMSNorm (ssq from the bf16 latents; negligible vs fp32)
    rstd_q = 1.0 / np.sqrt((qa_full ** 2).mean(1) + EPS)
    rstd_kv = 1.0 / np.sqrt((kva_full[:, :KV_RANK] ** 2).mean(1) + EPS)

    qa_n = qa_full * rstd_q[:, None]
    kva_n = kva_full[:, :KV_RANK] * rstd_kv[:, None]
    kpe_raw = kva_full[:, KV_RANK:]  # [T, 64], not normalized

    # host rope for k_pe (shared across heads); rows de-interleaved evens|odds
    x1, x2 = kpe_raw[:, 0::2], kpe_raw[:, 1::2]
    kx = x1 * cos - x2 * sin
    ky = x2 * cos + x1 * sin
    kpeT = np.concatenate([kx.T, ky.T], 0)          # [64, T]
    kpe2_host = np.ascontiguousarray(
        np.concatenate([kpeT, kpeT], 0).astype(BFNP))  # [128, T]

    qaT_p = np.ascontiguousarray(
        qa_n.T.reshape(RCH, 128, T).transpose(1, 0, 2).astype(BFNP))
    kvaT_p = np.ascontiguousarray(
        kva_n.T.reshape(KVCH, 128, T).transpose(1, 0, 2).astype(BFNP))

    # q-rope tables: CS = cos tiled 4x, SS = [-s, +s, -s, +s]
    cos_t = cos.T  # [32, T]
    sin_t = sin.T
    cs_host = np.ascontiguousarray(
        np.concatenate([cos_t] * 4, 0).astype(BFNP))
    ss_host = np.ascontiguousarray(
        np.concatenate([-sin_t, sin_t, -sin_t, sin_t], 0).astype(BFNP))

    cols = np.arange(896) - 384
    bigmask = np.ascontiguousarray(
        (cols[None, :] >= np.arange(128)[:, None]).astype(np.float16))

    # fold RMSNorm weights into the up-projection weights
    wq_b_eff = (wq_b * q_a_norm_w[:, None]).reshape(Q_RANK, H, D_QK)
    wkv_b_eff = (wkv_b * kv_a_norm_w[:, None]).reshape(KV_RANK, H, D_NOPE + D_V)
    wo_r = wo.reshape(H, D_V, HID)

    def pack_stat(w):  # [K*128, M] -> [128, K, M]
        kch = w.shape[0] // 128
        return np.ascontiguousarray(
            w.reshape(kch, 128, w.shape[1]).transpose(1, 0, 2).astype(BFNP))

    # ---------------- launch 2 ----------------
    nc2 = _get_built("p2")
    in_maps2 = []
    for c in range(N_CORES):
        heads = [c * HPC, c * HPC + 1]
        wqbn_w = wq_b_eff[:, heads, :D_NOPE].reshape(Q_RANK, HPC * 128)
        pe = wq_b_eff[:, heads, D_NOPE:]            # [R, 2, 64]
        px, py = pe[:, :, 0::2], pe[:, :, 1::2]     # [R, 2, 32]
        wqbp_w = np.concatenate(
            [px[:, 0], py[:, 0], px[:, 1], py[:, 1]], 1)  # [R, 128] order C
        wkbn_w = wkv_b_eff[:, heads, :D_NOPE].reshape(KV_RANK, HPC * 128)
        wkbv_w = wkv_b_eff[:, heads, D_NOPE:].reshape(KV_RANK, HPC * 128)
        in_maps2.append({
            "kvaTp": kvaT_p,
            "qaTp": qaT_p,
            "kpe2": kpe2_host,
            "wkbn": np.ascontiguousarray(
                pack_stat(wkbn_w).reshape(128, KVCH, HPC, 128)),
            "wkbv": pack_stat(wkbv_w),
            "wqbn": np.ascontiguousarray(
                pack_stat(wqbn_w).reshape(128, RCH, HPC, 128)),
            "wqbp": pack_stat(wqbp_w),
            "wop": np.ascontiguousarray(
                wo_r[heads].transpose(1, 0, 2).astype(BFNP)),
            "csd": cs_host,
            "ssd": ss_host,
            "maskd": bigmask,
        })
    res2 = run_bass_kernel_spmd(nc2, in_maps2, core_ids=list(range(N_CORES)),
                                trace=trace)
    if trace:
        LAST_EXEC_NS.append(res2.exec_time_ns)

    out = np.zeros((T, HID), np.float64)
    for c in range(N_CORES):
        out += res2.results[c]["out_p"].astype(np.float64)
    return out.astype(np.float32)


# revision 23
# speedup vs baseline: 1.0180x; 1.0180x over previous
# DeepseekV2 MLA attention (T=2048, H=16) on 8 TRN2 NeuronCores.
#
# Two launches (host gather/transpose/normalize between them is free):
#   Launch 1 (seq x col 2D, 4 seq-blocks x 2 weight-col-halves): each core
#     computes raw low-rank latents for its 512-token block and weight-column
#     half.  RMSNorm (incl. sum-of-squares from the bf16 latents) + k_pe rope
#     are applied on the host (elementwise, cheap).
#   Launch 2 (head-parallel, 2 heads/core): up-projections, q-rope, causal
#     softmax attention (scores kept [k, q]; denominator via DVE column-sum
#     accumulation + one fp32 broadcast matmul), o_proj partial in bf16;
#     host sums the 8 partials.
#
# Perf notes vs the naive version:
#   - all DRAM tensors host-packed so each DMA descriptor covers >=2KB/partition
#     (the 16 DMA queues are descriptor-rate-bound at ~95ns/descriptor)
#   - PE kept continuously busy (TRN2 PE needs ~3us of back-to-back work to
#     reach the 2.4GHz p-state)
#   - no per-tile softmax-denominator matmuls; no 1-partition reciprocals
#   - causal diagonal tiles narrowed to valid columns
#   - both heads' rope projection packed into one stationary; rope applied by
#     DVE with a DMA partition-block swap
#   - RMSNorm weights folded into wq_b / wkv_b on the host
import contextlib
import ctypes
import math
import sys
import types

import numpy as np

# ---------------------------------------------------------------- constants
H = 16
D_NOPE = 128
D_ROPE = 64
D_QK = D_NOPE + D_ROPE
D_V = 128
HID = 2048
Q_RANK = 1536
KV_RANK = 512
EPS = 1e-6
T = 2048
BASE = 10000.0
FACTOR = 40.0
ORIG_MAX = 4096
BETA_FAST = 32.0
BETA_SLOW = 1.0
MSCALE = 0.707
MSCALE_ALL = 0.707

N_CORES = 8
SEQB = 4                    # launch-1 sequence blocks
TC1 = T // SEQB             # 512 tokens per launch-1 core
WQH = Q_RANK // 2           # 768 wq_a columns per half
WKH = (KV_RANK + D_ROPE) // 2  # 288 wkv_a columns per half
HPC = H // N_CORES          # 2 heads per launch-2 core
QT = 512                    # q-tile width
KT = 128                    # k-tile height
RCH = Q_RANK // 128         # 12
KVCH = KV_RANK // 128       # 4
KCH = HID // 128            # 16


def _yarn_mscale(scale, mscale):
    return 1.0 if scale <= 1 else 0.1 * mscale * math.log(scale) + 1.0


SCALING = D_QK ** -0.5 * _yarn_mscale(FACTOR, MSCALE_ALL) ** 2

# ------------------------------------------------------- NTFF profiling shim
LAST_EXEC_NS = []


def _install_ntff_shim():
    try:
        import antenv.axon_hooks  # noqa: F401
        return
    except ImportError:
        pass
    try:
        so_path = "/opt/axon/libaxon_pjrt.so"
        lib = ctypes.CDLL(so_path)
        if not hasattr(lib, "axon_start_nrt_profile"):
            hook = None
        else:
            lib.axon_start_nrt_profile.argtypes = [
                ctypes.POINTER(ctypes.c_int64),
                ctypes.c_size_t,
            ]
            lib.axon_start_nrt_profile.restype = ctypes.c_int64
            lib.axon_stop_nrt_profile.argtypes = [ctypes.c_char_p]
            lib.axon_stop_nrt_profile.restype = ctypes.c_int64

            @contextlib.contextmanager
            def hook(output_dir, device_ids):
                import jax

                jax.devices()
                if device_ids:
                    ids = (ctypes.c_int64 * len(device_ids))(*device_ids)
                    rc = lib.axon_start_nrt_profile(ids, len(device_ids))
                else:
                    rc = lib.axon_start_nrt_profile(None, 0)
                if rc != 0:
                    raise RuntimeError(f"axon_start_nrt_profile rc={rc}")
                try:
                    yield
                finally:
                    n = lib.axon_stop_nrt_profile(str(output_dir).encode())
                    if n < 0:
                        raise RuntimeError(f"axon_stop_nrt_profile rc={n}")

        mod = types.ModuleType("antenv.axon_hooks")
        mod.get_axon_ntff_profile_hook = lambda: hook
        mod.set_axon_ntff_profile_hook = lambda h: None
        sys.modules["antenv.axon_hooks"] = mod
    except Exception:
        pass


_install_ntff_shim()

# ------------------------------------------------------------- host helpers


def _rope_tables(positions):
    dim = D_ROPE
    pos_freqs = BASE ** (np.arange(0, dim, 2, dtype=np.float64) / dim)
    inv_extra = 1.0 / pos_freqs
    inv_inter = 1.0 / (FACTOR * pos_freqs)

    def corr(nr):
        return dim * math.log(ORIG_MAX / (nr * 2 * math.pi)) / (2 * math.log(BASE))

    low = max(math.floor(corr(BETA_FAST)), 0)
    high = min(math.ceil(corr(BETA_SLOW)), dim - 1)
    ramp = np.clip(
        (np.arange(dim // 2, dtype=np.float64) - low) / max(high - low, 0.001), 0.0, 1.0
    )
    mask = 1.0 - ramp
    inv_freq = inv_inter * (1.0 - mask) + inv_extra * mask
    freqs = np.outer(np.asarray(positions, np.float64), inv_freq)
    m = _yarn_mscale(FACTOR, MSCALE) / _yarn_mscale(FACTOR, MSCALE_ALL)
    return (np.cos(freqs) * m).astype(np.float32), (np.sin(freqs) * m).astype(np.float32)


# ------------------------------------------------------------ bass builders
_BUILD_CACHE = {}


def _build_phase1():
    from concourse import bacc, mybir
    from concourse.tile import TileContext

    F32 = mybir.dt.float32
    BF16 = mybir.dt.bfloat16

    MT1 = TC1 // 128
    nc = bacc.Bacc()
    hTp = nc.dram_tensor("hTp", [128, KCH, TC1], BF16, kind="ExternalInput")
    wqap = nc.dram_tensor("wqap", [128, KCH, WQH], BF16, kind="ExternalInput")
    wkvap = nc.dram_tensor("wkvap", [128, KCH, WKH], BF16, kind="ExternalInput")
    qa_out = nc.dram_tensor("qa", [128, MT1, WQH], BF16, kind="ExternalOutput")
    kva_out = nc.dram_tensor("kva", [128, MT1, WKH], BF16, kind="ExternalOutput")

    MT = TC1 // 128  # 4 token tiles

    with TileContext(nc) as tc, contextlib.ExitStack() as ctx:
        pool = ctx.enter_context(tc.tile_pool(name="sb", bufs=1))
        work = ctx.enter_context(tc.tile_pool(name="wk", bufs=2))

        hT_sb = pool.tile([128, KCH, TC1], BF16, tag="hT")
        wqa_sb = pool.tile([128, KCH, WQH], BF16, tag="wqa")
        wkva_sb = pool.tile([128, KCH, WKH], BF16, tag="wkva")
        qa_st = pool.tile([128, MT, WQH], BF16, tag="qast")
        kva_st = pool.tile([128, MT, WKH], BF16, tag="kvast")

        # loads in consumption order; >=2KB/partition descriptors.
        # spread across engine rings for more in-flight descriptors
        def ld(out, in_):
            nc.sync.dma_start(out=out, in_=in_)

        for g in range(4):
            ks = slice(4 * g, 4 * g + 4)
            ld(hT_sb[:, ks, :], hTp[:, ks, :])
            ld(wqa_sb[:, ks, :], wqap[:, ks, :])
            if g in (1, 2):
                ks2 = slice(8 * (g - 1), 8 * (g - 1) + 8)
                ld(wkva_sb[:, ks2, :], wkvap[:, ks2, :])

        # qa + kva chains share one PSUM scope (4 + 4 banks) so there is no
        # pool barrier between the stages and the PE never goes idle
        with tc.tile_pool(name="ppq", bufs=1, space="PSUM") as ppq, \
             tc.tile_pool(name="ppk", bufs=2, space="PSUM") as ppk:
            for mg in range(2):
                qa_ps = [ppq.tile([128, WQH], F32, tag=f"qa{mi}",
                                  name=f"qa{mg}_{mi}") for mi in range(2)]
                for k in range(KCH):
                    for mi in range(2):
                        m = 2 * mg + mi
                        stat = hT_sb[:, k, m * 128:(m + 1) * 128]
                        nc.tensor.matmul(qa_ps[mi][:, 0:512], stat,
                                         wqa_sb[:, k, 0:512],
                                         start=(k == 0), stop=(k == KCH - 1))
                        nc.tensor.matmul(qa_ps[mi][:, 512:WQH], stat,
                                         wqa_sb[:, k, 512:WQH],
                                         start=(k == 0), stop=(k == KCH - 1))
                for mi in range(2):
                    m = 2 * mg + mi
                    nc.vector.tensor_copy(qa_st[:, m, :], qa_ps[mi][:, :])
                nc.sync.dma_start(out=qa_out[:, 2 * mg:2 * mg + 2, :],
                                  in_=qa_st[:, 2 * mg:2 * mg + 2, :])
            for mg in range(2):
                kv_ps = [ppk.tile([128, WKH], F32, tag=f"kv{mi}",
                                  name=f"kv{mg}_{mi}") for mi in range(2)]
                for k in range(KCH):
                    for mi in range(2):
                        m = 2 * mg + mi
                        stat = hT_sb[:, k, m * 128:(m + 1) * 128]
                        nc.tensor.matmul(kv_ps[mi][:, :], stat, wkva_sb[:, k, :],
                                         start=(k == 0), stop=(k == KCH - 1))
                for mi in range(2):
                    m = 2 * mg + mi
                    nc.vector.tensor_copy(kva_st[:, m, :], kv_ps[mi][:, :])
                nc.sync.dma_start(out=kva_out[:, 2 * mg:2 * mg + 2, :],
                                  in_=kva_st[:, 2 * mg:2 * mg + 2, :])

    nc.finalize()
    return nc


def _build_phase2():
    from concourse import bacc, mybir
    from concourse.tile import TileContext

    F32 = mybir.dt.float32
    BF16 = mybir.dt.bfloat16
    FP16 = mybir.dt.float16
    AF = mybir.ActivationFunctionType
    OP = mybir.AluOpType
    EXPB = -8.0 * math.log(2.0)  # exp bias; cancels in softmax, keeps fp16 range

    nc = bacc.Bacc()
    kvaTp = nc.dram_tensor("kvaTp", [128, KVCH, T], BF16, kind="ExternalInput")
    qaTp = nc.dram_tensor("qaTp", [128, RCH, T], BF16, kind="ExternalInput")
    kpe2 = nc.dram_tensor("kpe2", [128, T], BF16, kind="ExternalInput")
    wkbn = nc.dram_tensor("wkbn", [128, KVCH, HPC, 128], BF16, kind="ExternalInput")
    wkbv = nc.dram_tensor("wkbv", [128, KVCH, 256], BF16, kind="ExternalInput")
    wqbn = nc.dram_tensor("wqbn", [128, RCH, HPC, 128], BF16, kind="ExternalInput")
    wqbp = nc.dram_tensor("wqbp", [128, RCH, 128], BF16, kind="ExternalInput")
    wop = nc.dram_tensor("wop", [128, HPC, HID], BF16, kind="ExternalInput")
    csd = nc.dram_tensor("csd", [128, T], BF16, kind="ExternalInput")
    ssd = nc.dram_tensor("ssd", [128, T], BF16, kind="ExternalInput")
    maskd = nc.dram_tensor("maskd", [128, 896], FP16, kind="ExternalInput")
    out_p = nc.dram_tensor("out_p", [T, HID], BF16, kind="ExternalOutput")

    with TileContext(nc) as tc, contextlib.ExitStack() as ctx:
        persist = ctx.enter_context(tc.tile_pool(name="persist", bufs=1))

        kvaT_sb = persist.tile([128, KVCH, T], BF16, tag="kvaT")
        qaT_sb = persist.tile([128, RCH, T], BF16, tag="qaT")
        kpe2_sb = persist.tile([128, T], BF16, tag="kpe2")
        wkbn_sb = persist.tile([128, KVCH, HPC, 128], BF16, tag="wkbn")
        wkbv_sb = persist.tile([128, KVCH, 256], BF16, tag="wkbv")
        wqbn_sb = persist.tile([128, RCH, HPC, 128], BF16, tag="wqbn")
        wqbp_sb = persist.tile([128, RCH, 128], BF16, tag="wqbp")
        wo_sb = persist.tile([128, HPC, HID], BF16, tag="wo")
        cs_sb = persist.tile([128, T], BF16, tag="cs")
        ss_sb = persist.tile([128, T], BF16, tag="ss")
        mask_sb = persist.tile([128, 896], FP16, tag="mask")
        ones_sb = persist.tile([128, 128], FP16, tag="ones")
        nc.vector.memset(ones_sb[:, :], 1.0)
        expb_sb = persist.tile([128, 1], F32, tag="expb")
        nc.vector.memset(expb_sb[:, :], EXPB)

        knopeT = [persist.tile([128, T], BF16, tag=f"knopeT{h}", name=f"knopeT{h}") for h in range(HPC)]
        v_nat = [persist.tile([128, T], FP16, tag=f"vnat{h}", name=f"vnat{h}") for h in range(HPC)]
        qnT = [persist.tile([128, T], BF16, tag=f"qnT{h}", name=f"qnT{h}") for h in range(HPC)]
        qpeT = persist.tile([128, T], BF16, tag="qpeT")  # [h0 x'|y' ; h1 x'|y']
        aoT = [persist.tile([128, T], BF16, tag=f"aoT{h}", name=f"aoT{h}") for h in range(HPC)]
        colsum = [persist.tile([128, QT], FP16, tag=f"colsum{h}", name=f"colsum{h}") for h in range(HPC)]

        # loads: PE-critical order first
        def ld(out, in_):
            nc.sync.dma_start(out=out, in_=in_)

        ld(kvaT_sb[:, 0, :], kvaTp[:, 0, :])
        ld(wkbn_sb[:, :, :, :], wkbn[:, :, :, :])
        ld(wkbv_sb[:, :, :], wkbv[:, :, :])
        for k in range(1, KVCH):
            ld(kvaT_sb[:, k, :], kvaTp[:, k, :])
        ld(wqbn_sb[:, :, :, :], wqbn[:, :, :, :])
        ld(wqbp_sb[:, :, :], wqbp[:, :, :])
        for k in range(RCH):
            ld(qaT_sb[:, k, :], qaTp[:, k, :])
        ld(cs_sb[:, :], csd[:, :])
        ld(ss_sb[:, :], ssd[:, :])
        ld(kpe2_sb[:, :], kpe2[:, :])
        ld(mask_sb[:, :], maskd[:, :])
        ld(wo_sb[:, :, :], wop[:, :, :])

        # ---------------- stage 1a: k_nope^T = wkbn^T kva, k-outer ----------
        with tc.tile_pool(name="ppkn", bufs=1, space="PSUM") as ppkn:
            kn_ps = [[ppkn.tile([128, 512], F32, tag=f"kn{h}_{n}", name=f"kn{h}_{n}")
                      for n in range(4)] for h in range(HPC)]
            for k in range(KVCH):
                for h in range(HPC):
                    for n in range(4):
                        nsl = slice(n * 512, (n + 1) * 512)
                        nc.tensor.matmul(
                            kn_ps[h][n][:, :], wkbn_sb[:, k, h, :],
                            kvaT_sb[:, k, nsl],
                            start=(k == 0), stop=(k == KVCH - 1))
            for h in range(HPC):
                for n in range(4):
                    nsl = slice(n * 512, (n + 1) * 512)
                    if n % 2 == 0:
                        nc.scalar.copy(knopeT[h][:, nsl], kn_ps[h][n][:, :])
                    else:
                        nc.vector.tensor_copy(knopeT[h][:, nsl], kn_ps[h][n][:, :])

        # ---------------- stage 1b: v (both heads packed) -------------------
        with tc.tile_pool(name="ppv", bufs=4, space="PSUM") as ppv:
            for t in range(T // 128):
                tsl = slice(t * 128, (t + 1) * 128)
                v_ps = ppv.tile([128, 256], F32, tag="v", name=f"v{t}")
                for k in range(KVCH):
                    nc.tensor.matmul(v_ps[:, :], kvaT_sb[:, k, tsl],
                                     wkbv_sb[:, k, :],
                                     start=(k == 0), stop=(k == KVCH - 1))
                nc.scalar.copy(v_nat[0][:, tsl], v_ps[:, 0:128])
                nc.scalar.copy(v_nat[1][:, tsl], v_ps[:, 128:256])

        # ------------- stage 2: q up-projections + rope (2-qtr groups) ------
        with tc.tile_pool(name="ppg", bufs=2, space="PSUM") as ppg, \
             tc.tile_pool(name="qwork", bufs=2) as qwork:
            for qg in range(2):
                qtrs = [2 * qg, 2 * qg + 1]
                qn_ps = {}
                qp_ps = {}
                for qtr in qtrs:
                    qn_ps[qtr] = [ppg.tile([128, QT], F32, tag=f"qn{h}",
                                           name=f"qn{h}_{qtr}")
                                  for h in range(HPC)]
                    qp_ps[qtr] = ppg.tile([128, QT], F32, tag="qp",
                                          name=f"qp_{qtr}")
                for k in range(RCH):
                    for qtr in qtrs:
                        qsl = slice(qtr * QT, (qtr + 1) * QT)
                        mov = qaT_sb[:, k, qsl]
                        for h in range(HPC):
                            nc.tensor.matmul(qn_ps[qtr][h][:, :],
                                             wqbn_sb[:, k, h, :], mov,
                                             start=(k == 0), stop=(k == RCH - 1))
                        nc.tensor.matmul(qp_ps[qtr][:, :], wqbp_sb[:, k, :], mov,
                                         start=(k == 0), stop=(k == RCH - 1))
                for qtr in qtrs:
                    qsl = slice(qtr * QT, (qtr + 1) * QT)
                    for h in range(HPC):
                        nc.scalar.copy(qnT[h][:, qsl], qn_ps[qtr][h][:, :])
                    # rope: evacuate, block-swap head-halves via DMA, then
                    # qpeT = qp*CS + swap(qp)*SS on DVE
                    qp_sb = qwork.tile([128, QT], F32, tag="qpsb",
                                       name=f"qpsb{qtr}")
                    nc.scalar.copy(qp_sb[:, :], qp_ps[qtr][:, :])
                    sw_sb = qwork.tile([128, QT], F32, tag="swsb",
                                       name=f"swsb{qtr}")
                    nc.gpsimd.dma_start(out=sw_sb[0:32, :], in_=qp_sb[32:64, :])
                    nc.gpsimd.dma_start(out=sw_sb[32:64, :], in_=qp_sb[0:32, :])
                    nc.gpsimd.dma_start(out=sw_sb[64:96, :], in_=qp_sb[96:128, :])
                    nc.gpsimd.dma_start(out=sw_sb[96:128, :], in_=qp_sb[64:96, :])
                    ta = qwork.tile([128, QT], BF16, tag="ta", name=f"ta{qtr}")
                    nc.vector.tensor_tensor(ta[:, :], qp_sb[:, :], cs_sb[:, qsl],
                                            op=OP.mult)
                    tb = qwork.tile([128, QT], BF16, tag="tb", name=f"tb{qtr}")
                    nc.vector.tensor_tensor(tb[:, :], sw_sb[:, :], ss_sb[:, qsl],
                                            op=OP.mult)
                    nc.vector.tensor_tensor(qpeT[:, qsl], ta[:, :], tb[:, :],
                                            op=OP.add)

        # ------- stage 3: attention; softmax tail + o_proj pipelined 1 qtr ----
        with tc.tile_pool(name="pps", bufs=2, space="PSUM") as pps, \
             tc.tile_pool(name="ppu", bufs=3, space="PSUM") as ppu, \
             tc.tile_pool(name="ppd", bufs=1, space="PSUM") as ppd, \
             tc.tile_pool(name="ppo", bufs=2, space="PSUM") as ppo, \
             tc.tile_pool(name="awork", bufs=8) as awork, \
             tc.tile_pool(name="rwork", bufs=2) as rwork, \
             tc.tile_pool(name="owork", bufs=3) as owork:

            o_sb_map = {}

            def emit_o_unit(qtr, tt, j):
                # one o_proj 512-col chunk for token tile tt of q-range qtr
                q0o = qtr * QT
                tslo = slice(q0o + tt * 128, q0o + (tt + 1) * 128)
                jsl = slice(j * 512, (j + 1) * 512)
                if j == 0:
                    o_sb_map[(qtr, tt)] = owork.tile(
                        [128, HID], BF16, tag="osb", name=f"o{qtr}_{tt}")
                o_sb = o_sb_map[(qtr, tt)]
                o_ps = ppo.tile([128, 512], F32, tag="o",
                                name=f"op{qtr}_{tt}_{j}")
                for h in range(HPC):
                    nc.tensor.matmul(o_ps[:, :], aoT[h][:, tslo],
                                     wo_sb[:, h, jsl],
                                     start=(h == 0), stop=(h == HPC - 1))
                if j % 2 == 0:
                    nc.scalar.copy(o_sb[:, jsl], o_ps[:, :])
                else:
                    nc.vector.tensor_copy(o_sb[:, jsl], o_ps[:, :])
                if j == 3:
                    nc.sync.dma_start(out=out_p[tslo, :], in_=o_sb[:, :])

            def make_tail(qtr, h, un_t):
                def emit():
                    qslh = slice(qtr * QT, (qtr + 1) * QT)
                    denb_ps = ppd.tile([128, QT], F32, tag="denb",
                                       name=f"db{h}_{qtr}")
                    nc.tensor.matmul(denb_ps[:, :], ones_sb[:, :],
                                     colsum[h][:, :], start=True, stop=True)
                    recip = rwork.tile([128, QT], F32, tag="recip",
                                       name=f"r{h}_{qtr}")
                    nc.vector.reciprocal_approx_fast(out=recip[:, :],
                                                     in_=denb_ps[:, :])
                    nc.vector.tensor_tensor(aoT[h][:, qslh], un_t[:, :],
                                            recip[:, :], op=OP.mult)
                return emit

            filler = []
            for qtr in range(T // QT):
                q0 = qtr * QT
                n_k = (q0 + QT) // KT
                un_ps = [None] * HPC
                for ki in range(n_k):
                    k0 = ki * KT
                    ksl = slice(k0, k0 + KT)
                    d = k0 - q0
                    coff = max(d, 0)
                    w = QT - coff
                    msl = slice(q0 + coff, q0 + QT)
                    for h in range(HPC):
                        if filler:
                            filler.pop(0)()
                        if ki == 0:
                            un_ps[h] = ppu.tile([128, QT], F32, tag="un",
                                                name=f"un{h}_{qtr}")
                        hb = slice(64 * h, 64 * h + 64)
                        s_ps = pps.tile([128, QT], F32, tag="s",
                                        name=f"s{h}_{qtr}_{ki}")
                        nc.tensor.matmul(s_ps[:, 0:w], knopeT[h][:, ksl],
                                         qnT[h][:, msl], start=True, stop=False)
                        nc.tensor.matmul(s_ps[:, 0:w], kpe2_sb[hb, ksl],
                                         qpeT[hb, msl], start=False, stop=True)
                        expT = awork.tile([128, QT], FP16, tag="expT",
                                          name=f"e{h}_{qtr}_{ki}")
                        nc.scalar.activation(out=expT[:, 0:w], in_=s_ps[:, 0:w],
                                             func=AF.Exp, scale=SCALING,
                                             bias=expb_sb[:, :])
                        if d >= 0:
                            nc.vector.tensor_tensor(
                                expT[:, 0:w], expT[:, 0:w],
                                mask_sb[:, 384:384 + w], op=OP.mult)
                        if ki == 0:
                            nc.vector.tensor_copy(colsum[h][:, :], expT[:, :])
                        else:
                            nc.vector.tensor_tensor(
                                colsum[h][:, coff:QT], colsum[h][:, coff:QT],
                                expT[:, 0:w], op=OP.add)
                        nc.tensor.matmul(un_ps[h][:, coff:QT], v_nat[h][:, ksl],
                                         expT[:, 0:w],
                                         start=(ki == 0), stop=(ki == n_k - 1),
                                         skip_group_check=True)
                while filler:
                    filler.pop(0)()
                filler = [make_tail(qtr, h, un_ps[h]) for h in range(HPC)]
                filler += [(lambda a, b, c: (lambda: emit_o_unit(a, b, c)))(
                    qtr, tt, j) for tt in range(QT // 128) for j in range(4)]
            while filler:
                filler.pop(0)()

    nc.finalize()
    return nc


def _get_built(name):
    if name not in _BUILD_CACHE:
        _BUILD_CACHE[name] = _build_phase1() if name == "p1" else _build_phase2()
    return _BUILD_CACHE[name]


# ---------------------------------------------------------------- kernel()


def kernel(positions, hidden_states, wq_a, q_a_norm_w, wq_b, wkv_a, kv_a_norm_w,
           wkv_b, wo):
    import os

    from concourse.bass_utils import run_bass_kernel_spmd
    import ml_dtypes

    BFNP = ml_dtypes.bfloat16
    trace = bool(os.environ.get("BASS_KERNEL_TRACE"))
    LAST_EXEC_NS.clear()

    positions = np.asarray(positions)
    hidden = np.asarray(hidden_states, np.float32)
    wq_a = np.asarray(wq_a, np.float32)
    wq_b = np.asarray(wq_b, np.float32)
    wkv_a = np.asarray(wkv_a, np.float32)
    wkv_b = np.asarray(wkv_b, np.float32)
    wo = np.asarray(wo, np.float32)
    q_a_norm_w = np.asarray(q_a_norm_w, np.float32)
    kv_a_norm_w = np.asarray(kv_a_norm_w, np.float32)

    cos, sin = _rope_tables(positions)  # [T, 32] f32

    # ---------------- launch 1: latents (4 seq blocks x 2 col halves) -------
    hidden_bf = hidden.astype(BFNP)
    wqa_halves = []
    wkva_halves = []
    for half in range(2):
        wq_h = wq_a[:, half * WQH:(half + 1) * WQH].astype(BFNP)
        wqa_halves.append(np.ascontiguousarray(
            wq_h.reshape(KCH, 128, WQH).transpose(1, 0, 2)))
        wk_h = wkv_a[:, half * WKH:(half + 1) * WKH].astype(BFNP)
        wkva_halves.append(np.ascontiguousarray(
            wk_h.reshape(KCH, 128, WKH).transpose(1, 0, 2)))

    in_maps1 = []
    for c in range(N_CORES):
        seq, half = c // 2, c % 2
        hs = hidden_bf[seq * TC1:(seq + 1) * TC1]  # [512, 2048]
        hTp = np.ascontiguousarray(hs.reshape(TC1, KCH, 128).transpose(2, 1, 0))
        in_maps1.append({
            "hTp": hTp,
            "wqap": wqa_halves[half],
            "wkvap": wkva_halves[half],
        })

    nc1 = _get_built("p1")
    res1 = run_bass_kernel_spmd(nc1, in_maps1, core_ids=list(range(N_CORES)),
                                trace=trace)
    if trace:
        LAST_EXEC_NS.append(res1.exec_time_ns)

    qa_full = np.empty((T, Q_RANK), np.float32)
    kva_full = np.empty((T, KV_RANK + D_ROPE), np.float32)
    for c in range(N_CORES):
        seq, half = c // 2, c % 2
        r = res1.results[c]
        tsl = slice(seq * TC1, (seq + 1) * TC1)
        qa_full[tsl, half * WQH:(half + 1) * WQH] = (
            np.asarray(r["qa"], np.float32).transpose(1, 0, 2).reshape(TC1, WQH))
        kva_full[tsl, half * WKH:(half + 1) * WKH] = (
            np.asarray(r["kva"], np.float32).transpose(1, 0, 2).reshape(TC1, WKH))

    # host RMSNorm (ssq from the bf16 latents; negligible vs fp32)
    rstd_q = 1.0 / np.sqrt((qa_full ** 2).mean(1) + EPS)
    rstd_kv = 1.0 / np.sqrt((kva_full[:, :KV_RANK] ** 2).mean(1) + EPS)

    qa_n = qa_full * rstd_q[:, None]
    kva_n = kva_full[:, :KV_RANK] * rstd_kv[:, None]
    kpe_raw = kva_full[:, KV_RANK:]  # [T, 64], not normalized

    # host rope for k_pe (shared across heads); rows de-interleaved evens|odds
    x1, x2 = kpe_raw[:, 0::2], kpe_raw[:, 1::2]
    kx = x1 * cos - x2 * sin
    ky = x2 * cos + x1 * sin
    kpeT = np.concatenate([kx.T, ky.T], 0)          # [64, T]
    kpe2_host = np.ascontiguousarray(
        np.concatenate([kpeT, kpeT], 0).astype(BFNP))  # [128, T]

    qaT_p = np.ascontiguousarray(
        qa_n.T.reshape(RCH, 128, T).transpose(1, 0, 2).astype(BFNP))
    kvaT_p = np.ascontiguousarray(
        kva_n.T.reshape(KVCH, 128, T).transpose(1, 0, 2).astype(BFNP))

    # q-rope tables: CS = cos tiled 4x, SS = [-s, +s, -s, +s]
    cos_t = cos.T  # [32, T]
    sin_t = sin.T
    cs_host = np.ascontiguousarray(
        np.concatenate([cos_t] * 4, 0).astype(BFNP))
    ss_host = np.ascontiguousarray(
        np.concatenate([-sin_t, sin_t, -sin_t, sin_t], 0).astype(BFNP))

    cols = np.arange(896) - 384
    bigmask = np.ascontiguousarray(
        (cols[None, :] >= np.arange(128)[:, None]).astype(np.float16))

    # fold RMSNorm weights into the up-projection weights
    wq_b_eff = (wq_b * q_a_norm_w[:, None]).reshape(Q_RANK, H, D_QK)
    wkv_b_eff = (wkv_b * kv_a_norm_w[:, None]).reshape(KV_RANK, H, D_NOPE + D_V)
    wo_r = wo.reshape(H, D_V, HID)

    def pack_stat(w):  # [K*128, M] -> [128, K, M]
        kch = w.shape[0] // 128
        return np.ascontiguousarray(
            w.reshape(kch, 128, w.shape[1]).transpose(1, 0, 2).astype(BFNP))

    # ---------------- launch 2 ----------------
    nc2 = _get_built("p2")
    in_maps2 = []
    for c in range(N_CORES):
        heads = [c * HPC, c * HPC + 1]
        wqbn_w = wq_b_eff[:, heads, :D_NOPE].reshape(Q_RANK, HPC * 128)
        pe = wq_b_eff[:, heads, D_NOPE:]            # [R, 2, 64]
        px, py = pe[:, :, 0::2], pe[:, :, 1::2]     # [R, 2, 32]
        wqbp_w = np.concatenate(
            [px[:, 0], py[:, 0], px[:, 1], py[:, 1]], 1)  # [R, 128] order C
        wkbn_w = wkv_b_eff[:, heads, :D_NOPE].reshape(KV_RANK, HPC * 128)
        wkbv_w = wkv_b_eff[:, heads, D_NOPE:].reshape(KV_RANK, HPC * 128)
        in_maps2.append({
            "kvaTp": kvaT_p,
            "qaTp": qaT_p,
            "kpe2": kpe2_host,
            "wkbn": np.ascontiguousarray(
                pack_stat(wkbn_w).reshape(128, KVCH, HPC, 128)),
            "wkbv": pack_stat(wkbv_w),
            "wqbn": np.ascontiguousarray(
                pack_stat(wqbn_w).reshape(128, RCH, HPC, 128)),
            "wqbp": pack_stat(wqbp_w),
            "wop": np.ascontiguousarray(
                wo_r[heads].transpose(1, 0, 2).astype(BFNP)),
            "csd": cs_host,
            "ssd": ss_host,
            "maskd": bigmask,
        })
    res2 = run_bass_kernel_spmd(nc2, in_maps2, core_ids=list(range(N_CORES)),
                                trace=trace)
    if trace:
        LAST_EXEC_NS.append(res2.exec_time_ns)

    out = np.zeros((T, HID), np.float64)
    for c in range(N_CORES):
        out += res2.results[c]["out_p"].astype(np.float64)
    return out.astype(np.float32)
